# revision 2
# baseline (speedup 1.0000x reference)
"""Trainium2 Bass kernel for a 2-layer LSTM (H=64) + FC head. V3.

Problem: x [4096, 168, 19] f32 -> out [4096] f32
  h1 = LSTM0(x); h2 = LSTM1(h1); out = h2[:, -1, :] @ Wfc.T + bfc

Data-parallel over 8 NeuronCores (512 batch rows each); CH=3 chains
(171/171/170 rows) per core whose recurrences interleave. Layer 0 at
time w and layer 1 at time w-1 share each wave (p0:64 = L0, p64:128 =
L1).

The per-wave period is bounded by each chain's serial loop
(mm -> acts -> cell ops -> tanh(c) -> h -> mm), so smaller chains
shorten the loop; the ACT engine's busy time (el work + per-inst
init) bounds it from below. CH=3 balances the two. Matmuls run in
bf16 (fp32r needs N>=256; bf16 is 1 cycle/row at any N).

  z PSUM tile [128, 4banks, cb] per chain: banks = G, F, I, O.
  The G-gate weights are pre-scaled by 2 so ONE sigmoid over all four
  banks yields G' = sigmoid(2 z_g) with tanh(z_g) = 2G'-1; the Pool
  fixup computes u = i*g = (G'*2)*I - I in two ops.
  ACT (in-place over PSUM): sigmoid(G',F,I,O); tanh(c')->TC.
  Pool: t2 = (G'*2)*I (into G bank); u = t2-I (into I bank);
        v = F*c (into F bank); c' = v+u; h = OC*TC -> hm (bf16, SBUF).
  DVE: copies O out of Z (frees Z early for next wave's x-part mms).
  C, TC, OC in SBUF f32 (PSUM has room only for the six z banks).
"""

import numpy as np

HIDDEN = 64
INPUT = 19
B = 4096
T = 168
NCORES = 8
BL = B // NCORES   # 512 per core
CBS = [171, 171, 170]
CH = len(CBS)
OFFS = [0, 171, 342]
H4 = 4 * HIDDEN    # 256

# torch gate order rows: i(0:64) f(64:128) g(128:192) o(192:256)
# our bank (column-block) order: G, F, I, O
GATE_PERM = np.concatenate([
    np.arange(128, 192),  # g
    np.arange(64, 128),   # f
    np.arange(0, 64),     # i
    np.arange(192, 256),  # o
])


def build_nc(steps=T):
    import concourse.bacc as bacc
    import concourse.tile as tile
    from concourse import mybir

    F32 = mybir.dt.float32
    BF16 = mybir.dt.bfloat16
    AF = mybir.ActivationFunctionType

    nc = bacc.Bacc("TRN2", target_bir_lowering=False, debug=False,
                   num_devices=NCORES)

    xT = nc.dram_tensor("xT", [T, INPUT + 1, BL], BF16, kind="ExternalInput")
    w0x_d = nc.dram_tensor("w0x", [INPUT + 1, 512], BF16,
                           kind="ExternalInput")
    whbig_d = nc.dram_tensor("whbig", [128, 512], BF16, kind="ExternalInput")
    wfc_d = nc.dram_tensor("wfc", [128, 1], BF16, kind="ExternalInput")
    zeros_d = nc.dram_tensor("zeros", [128, max(CBS)], BF16,
                             kind="ExternalInput")
    out = nc.dram_tensor("out", [1, BL], F32, kind="ExternalOutput")

    with tile.TileContext(nc) as tc:
        with (
            tc.tile_pool(name="const", bufs=1) as const,
            tc.tile_pool(name="state", bufs=1) as state,
            tc.tile_pool(name="xin", bufs=6) as xin,
            tc.tile_pool(name="zpool", bufs=1, space="PSUM") as zpool,
            tc.tile_pool(name="fpool", bufs=1, space="PSUM") as fpool,
        ):
            w0x = const.tile([INPUT + 1, 4, 128], BF16, tag="w0x", name="w0x")
            whbig = const.tile([128, 4, 128], BF16, tag="wh", name="whbig")
            wfc = const.tile([128, 1], BF16, tag="wfc", name="wfc")
            nc.sync.dma_start(w0x, w0x_d[:])
            nc.sync.dma_start(whbig, whbig_d[:])
            nc.sync.dma_start(wfc, wfc_d[:])

            # z slots padded to 256 f32 so each matmul output stays
            # inside one half PSUM bank (outputs may not cross banks).
            Z = [zpool.tile([128, 4, 256], F32, tag=f"z{c}", name=f"z{c}")
                 for c in range(CH)]
            C = [[state.tile([128, CBS[c]], F32, tag=f"C{c}{p}",
                             name=f"C{c}{p}") for p in (0, 1)]
                 for c in range(CH)]
            TC = [state.tile([128, CBS[c]], F32, tag=f"TC{c}", name=f"TC{c}")
                  for c in range(CH)]
            OC = [state.tile([128, CBS[c]], F32, tag=f"OC{c}", name=f"OC{c}")
                  for c in range(CH)]
            hm = [[state.tile([128, CBS[c]], BF16, tag=f"hm{c}{p}",
                              name=f"hm{c}{p}") for p in (0, 1)]
                  for c in range(CH)]
            for c in range(CH):
                nc.gpsimd.memset(C[c][0], 0.0)
                nc.sync.dma_start(hm[c][0], zeros_d[:, :CBS[c]])

            nwaves = steps + 1

            def wave_body(w):
                cur, nxt = w % 2, (w + 1) % 2
                zfs = []
                for c in range(CH):
                    cb = CBS[c]
                    cs = slice(OFFS[c], OFFS[c] + cb)
                    xt = xin.tile([INPUT + 1, cb], BF16, tag=f"x{c}",
                                  name=f"x{c}")
                    nc.sync.dma_start(xt, xT[w % T, :, cs])
                    z = Z[c]
                    for b in range(4):
                        nc.tensor.matmul(z[:, b, 0:cb], w0x[:, b, :],
                                         xt[:], start=True,
                                         stop=False, skip_group_check=True)
                        nc.tensor.matmul(z[:, b, 0:cb], whbig[:, b, :],
                                         hm[c][cur][:], start=False,
                                         stop=True, skip_group_check=True)

                # One in-place sigmoid over all four banks (G-gate rows
                # pre-scaled by 2 host-side: tanh(zg) = 2*sigmoid(2zg)-1).
                for c in range(CH):
                    cb = CBS[c]
                    z = Z[c]
                    nc.scalar.activation(z[:, 0:4, 0:cb], z[:, 0:4, 0:cb],
                                         AF.Sigmoid)

                # Cell ops on Pool (in-place into gate banks); DVE copies
                # O out of Z in parallel.
                mult = mybir.AluOpType.mult
                for c in range(CH):
                    cb = CBS[c]
                    z = Z[c]
                    nc.gpsimd.scalar_tensor_tensor(
                        z[:, 0, 0:cb], z[:, 0, 0:cb], 2.0, z[:, 2, 0:cb],
                        mult, mult)
                    nc.gpsimd.tensor_sub(z[:, 2, 0:cb], z[:, 0, 0:cb],
                                         z[:, 2, 0:cb])
                    nc.gpsimd.tensor_mul(z[:, 1, 0:cb], z[:, 1, 0:cb],
                                         C[c][cur])
                    nc.gpsimd.tensor_add(C[c][nxt], z[:, 1, 0:cb],
                                         z[:, 2, 0:cb])
                    nc.vector.tensor_copy(OC[c], z[:, 3, 0:cb])

                # tanh(c') on ACT, then h = OC*tc -> hm (Pool)
                for c in range(CH):
                    nc.scalar.activation(TC[c], C[c][nxt], AF.Tanh)
                for c in range(CH):
                    nc.gpsimd.tensor_mul(hm[c][nxt], OC[c], TC[c])
                del zfs

                if w == 0:
                    # wave 0's layer-1 half ran on garbage; reset it
                    for c in range(CH):
                        nc.gpsimd.memset(C[c][nxt][64:128], 0.0)
                        nc.sync.dma_start(hm[c][nxt][64:128],
                                          zeros_d[64:128, :CBS[c]])

            for w in range(nwaves):
                wave_body(w)

            # --- FC head: out = Wfc . h1@steps-1 (bfc added on host) ---
            o_sb = state.tile([1, BL], F32, tag="osb", name="o_sb")
            for c in range(CH):
                pfc = fpool.tile([1, CBS[c]], F32, tag="pfc", name=f"pfc{c}")
                nc.tensor.matmul(pfc, wfc, hm[c][nwaves % 2][:],
                                 start=True, stop=True)
                nc.scalar.activation(o_sb[:, OFFS[c]:OFFS[c] + CBS[c]], pfc,
                                     AF.Copy)
            nc.sync.dma_start(out[:], o_sb)

    nc.compile()
    return nc


def make_in_maps(x, Wih0, Whh0, bih0, bhh0, Wih1, Whh1, bih1, bhh1, Wfc, bfc):
    """Shard + pre-transpose/concat inputs for the 8 cores."""
    p = GATE_PERM
    b0 = (bih0 + bhh0)[p].astype(np.float32)
    b1 = (bih1 + bhh1)[p].astype(np.float32)
    w0x = np.zeros((INPUT + 1, 4, 128), np.float32)
    whbig = np.zeros((128, 4, 128), np.float32)
    gs = np.ones((4, 1), np.float32)
    gs[0] = 2.0  # G-gate pre-scale: tanh(x) = 2*sigmoid(2x)-1
    for b in range(4):
        w0x[0:INPUT, b, 0:64] = Wih0[p].T[:, b * 64:(b + 1) * 64]
        w0x[INPUT, b, 0:64] = b0[b * 64:(b + 1) * 64]
        w0x[INPUT, b, 64:128] = b1[b * 64:(b + 1) * 64]
        whbig[0:64, b, 0:64] = Whh0[p].T[:, b * 64:(b + 1) * 64]
        whbig[0:64, b, 64:128] = Wih1[p].T[:, b * 64:(b + 1) * 64]
        whbig[64:128, b, 64:128] = Whh1[p].T[:, b * 64:(b + 1) * 64]
    wfcbig = np.zeros((128, 1), np.float32)
    wfcbig[64:128, 0] = Wfc.reshape(HIDDEN)

    def bf(a):
        import ml_dtypes
        return a.astype(ml_dtypes.bfloat16)

    w0x *= gs[None, :, :]
    whbig *= gs[None, :, :]
    base = {
        "w0x": bf(np.ascontiguousarray(w0x.reshape(INPUT + 1, 512))),
        "whbig": bf(np.ascontiguousarray(whbig.reshape(128, 512))),
        "wfc": bf(wfcbig),
        "zeros": bf(np.zeros((128, max(CBS)), np.float32)),
    }
    xs = x.reshape(NCORES, BL, T, INPUT)
    in_maps = []
    for c in range(NCORES):
        m = dict(base)
        xt = np.empty((T, INPUT + 1, BL), np.float32)
        xt[:, 0:INPUT, :] = xs[c].transpose(1, 2, 0)
        xt[:, INPUT, :] = 1.0
        m["xT"] = bf(xt)
        in_maps.append(m)
    return in_maps


_CACHED_NC = None


def kernel(**inputs):
    global _CACHED_NC
    from concourse.bass_utils import run_bass_kernel_spmd

    if _CACHED_NC is None:
        _CACHED_NC = build_nc()
    nc = _CACHED_NC
    in_maps = make_in_maps(**inputs)
    res = run_bass_kernel_spmd(nc, in_maps, list(range(NCORES)))
    outs = [res.results[c]["out"].reshape(BL) for c in range(NCORES)]
    return np.concatenate(outs) + np.float32(inputs["bfc"][0])
